# revision 23
# baseline (speedup 1.0000x reference)
"""MoE gate routing kernel (DeepSeek-V2-style group-limited top-k) for 8x TRN2 NeuronCores.

Problem: nn_MoEGate_13907104105110
  hidden_states [32768, 5120] fp32, gate weight [160, 5120] fp32
  logits = x @ W.T ; scores = softmax(logits)
  group-limited greedy top-k: 8 groups of 20 experts, keep top-3 groups by
  group max score, then top-6 scores of the kept groups, scaled by 16.0.
  Output: [32768, 6] fp32 (top-6 weights, descending).

Sharding: data-parallel over tokens; 4096 tokens per core, W replicated.

Modes:
  bf16x2p (default): host pre-transposes x and splits x,W into bf16 hi/lo.
    Per 128-token tile the device only does: one 2.62MB DMA, then per
    128-hidden chunk two matmuls with the x chunks stationary:
      MM1: xhi_k (stationary) x [Whi_k | Wlo_k]  -> PSUM[:, 0:320]   (N=320)
      MM2: xlo_k (stationary) x Whi_k            -> PSUM[:, 320:480] (N=160)
    logits = P[:,0:160] + P[:,160:320] + P[:,320:480]  (error ~2^-18, needed:
    pure-bf16 / fp32r logits flip near-ties in the top-k -> rel err ~1).
  bf16x3p: host pre-transposed, classic 3-MM hi/lo into one [160] region.
  bf16x3: on-device PE transpose variant (original baseline path).

Routing per tile (no max-subtraction: |logits| <~ 8 for this distribution, and
softmax selection is monotone so top-k is done on unnormalized exp scores):
  escore = Exp(logits) with accum ssum (ACT) -> rec = 1/ssum (DVE)
  group max over [128, 8, 20] -> top-8 -> 3rd value as group threshold
  -> mask groups (mult; escore > 0 so zeroing is safe) -> top-8 of masked
  -> out = top6 * rec * 16.0.
"""

import sys

if "/opt/trn_rl_repo" not in sys.path:
    sys.path.insert(0, "/opt/trn_rl_repo")

from contextlib import ExitStack

import ml_dtypes
import numpy as np

import concourse.bass as bass
import concourse.mybir as mybir
from concourse import bacc
from concourse import tile
from concourse.bass_utils import run_bass_kernel_spmd
from concourse.masks import make_identity

TOKENS = 32768
HIDDEN = 5120
NEXP = 160
TOPK = 6
NGROUP = 8
EPG = NEXP // NGROUP  # 20 experts per group
TOPK_GROUP = 3
SCALE = 16.0
NCORES = 8
TPC = TOKENS // NCORES  # 4096 tokens per core
PT = 128  # tokens per tile
KC = HIDDEN // 128  # 40 contraction chunks

F32 = mybir.dt.float32
BF16 = mybir.dt.bfloat16

MM_MODE = "bf16x2p"


def build_nc(tokens_per_core: int = TPC, mm_mode: str = MM_MODE, repeat: int = 1,
             hw_repeat: int = 1, bufs: dict | None = None,
             probe: str | None = None, psum_disjoint: bool = False,
             dma_split: bool = False, lo_n: int = NEXP) -> bass.Bass:
    # probe="dma": k-loop only does chunk 0 (measures DMA/routing side)
    # probe="pe":  every tile reuses tile 0's data (measures PE side)
    # psum_disjoint: lo@whi accumulates into its own PSUM region [320:480]
    # dma_split: issue hi/lo halves of the x tile on separate HWDGE rings
    B = {"x": 4, "xt": 2, "ps_tr": 4, "ps_lg": 4, "rt": 3, "st": 3}
    B.update(bufs or {})
    nt = tokens_per_core // PT
    nc = bacc.Bacc("TRN2", target_bir_lowering=False, debug=False)

    pretr = mm_mode in ("bf16x2p", "bf16x3p")
    if pretr:
        # host-pretransposed hi/lo: row t*128+p (p = hidden-in-chunk),
        # free dims [j(hi/lo), kchunk, token-in-tile]
        x_dram = nc.dram_tensor("x", [tokens_per_core, 2, KC, 128], BF16, kind="ExternalInput")
    else:
        x_dram = nc.dram_tensor("x", [tokens_per_core, HIDDEN], F32, kind="ExternalInput")
    if mm_mode == "bf16x2p":
        w_shape = [128, KC, 2 * NEXP]  # [Whi_k | Wlo_k] along free
    else:
        w_shape = [128, KC, 2, NEXP]
    w_dram = nc.dram_tensor("w", w_shape, BF16, kind="ExternalInput")
    out_dram = nc.dram_tensor("out", [tokens_per_core, TOPK], F32, kind="ExternalOutput")

    with tile.TileContext(nc) as tc, ExitStack() as ctx:
        const_pool = ctx.enter_context(tc.tile_pool(name="const", bufs=1))
        x_pool = ctx.enter_context(tc.tile_pool(name="x", bufs=B["x"]))
        ps_lg_pool = ctx.enter_context(tc.tile_pool(name="ps_lg", bufs=B["ps_lg"], space="PSUM"))
        rt_pool = ctx.enter_context(tc.tile_pool(name="rt", bufs=B["rt"]))
        st_pool = ctx.enter_context(tc.tile_pool(name="st", bufs=B["st"]))
        if not pretr:
            xt_pool = ctx.enter_context(tc.tile_pool(name="xt", bufs=B["xt"]))
            ps_tr_pool = ctx.enter_context(tc.tile_pool(name="ps_tr", bufs=B["ps_tr"], space="PSUM"))

        w_sb = const_pool.tile(w_shape, BF16)
        nc.sync.dma_start(w_sb[:], w_dram[:])
        if not pretr:
            ident = const_pool.tile([128, 128], F32)
            make_identity(nc, ident[:])

        xts0 = None
        if pretr and probe == "pe":
            xts0 = const_pool.tile([128, 2, KC, 128], BF16)
            nc.sync.dma_start(xts0[:], x_dram[0:PT, :, :, :])

        loop_ctx = ExitStack()
        if hw_repeat > 1:
            # hardware loop re-running the whole per-core pipeline; used by
            # test.py to amortize per-dispatch tunnel latency out of the
            # timing (each iteration is a complete kernel execution).
            loop_ctx.enter_context(
                tc.For_i(0, hw_repeat, 1,
                         staggered_reset=True,
                         hint_engines=(mybir.EngineType.PE, mybir.EngineType.DVE,
                                       mybir.EngineType.Activation, mybir.EngineType.SP))
            )
        ctx.enter_context(loop_ctx)

        for t in [i for _ in range(repeat) for i in range(nt)]:
            if pretr:
                if probe == "pe":
                    xts = xts0
                else:
                    xts = x_pool.tile([128, 2, KC, 128], BF16)
                    if dma_split:
                        nc.sync.dma_start(xts[:, 0], x_dram[t * PT : (t + 1) * PT, 0])
                        nc.scalar.dma_start(xts[:, 1], x_dram[t * PT : (t + 1) * PT, 1])
                    else:
                        nc.sync.dma_start(xts[:], x_dram[t * PT : (t + 1) * PT, :, :, :])
                xt_hi = xts[:, 0]
                xt_lo = xts[:, 1]
            else:
                x_sb = x_pool.tile([128, HIDDEN], F32)
                nc.sync.dma_start(x_sb[:], x_dram[t * PT : (t + 1) * PT, :])
                # transpose phase: x tile -> xT [128 hidden, KC, 128 tokens]
                xt_hi_t = xt_pool.tile([128, KC, 128], BF16, tag="xt_hi")
                xt_lo_t = xt_pool.tile([128, KC, 128], BF16, tag="xt_lo")
                GK = 4  # transposed chunks per PSUM bank; one batched copy per group
                for g in range(KC // GK):
                    xt_ps = ps_tr_pool.tile([128, GK, 128], F32)
                    for j in range(GK):
                        k = g * GK + j
                        nc.tensor.transpose(
                            xt_ps[:, j, :], x_sb[:, k * 128 : (k + 1) * 128], ident[:]
                        )
                    ks = slice(g * GK, (g + 1) * GK)
                    # hi = bf16(xT); lo = bf16(xT - hi)
                    nc.scalar.copy(xt_hi_t[:, ks, :], xt_ps[:])
                    nc.vector.tensor_sub(xt_lo_t[:, ks, :], xt_ps[:], xt_hi_t[:, ks, :])
                xt_hi = xt_hi_t[:]
                xt_lo = xt_lo_t[:]

            # matmul phase
            krange = [0] if probe == "dma" else list(range(KC))
            if mm_mode == "bf16x2p":
                # lg[:, 0:160] accumulates hi@whi; lg[:, 160:320] accumulates
                # hi@wlo (from the N=320 stream) PLUS lo@whi (second matmul
                # accumulating into the same region). PSUM accumulation is
                # per-element has_written on HW, so interleaved groups are
                # fine; skip_group_check silences the interp-only zero-region
                # bookkeeping which can't express overlapping groups.
                nl = 3 * NEXP if psum_disjoint else 2 * NEXP
                lg = ps_lg_pool.tile([128, nl], F32)
                for k in krange:
                    nc.tensor.matmul(lg[:, 0 : 2 * NEXP], xt_hi[:, k, :], w_sb[:, k, :],
                                     start=(k == 0), stop=(k == krange[-1]),
                                     skip_group_check=True)
                    if psum_disjoint:
                        nc.tensor.matmul(lg[:, 2 * NEXP : 3 * NEXP], xt_lo[:, k, :],
                                         w_sb[:, k, 0:NEXP],
                                         start=(k == 0), stop=(k == krange[-1]),
                                         skip_group_check=True)
                    else:
                        # lo stream width lo_n >= NEXP: columns beyond NEXP
                        # stream wlo and add genuine xlo@wlo terms into
                        # [NEXP:lo_n]; widths >=256 keep the stream long
                        # enough to hide the next chunk's LDWEIGHTS.
                        nc.tensor.matmul(lg[:, 0:lo_n], xt_lo[:, k, :],
                                         w_sb[:, k, 0:lo_n],
                                         start=False, stop=(k == krange[-1]),
                                         skip_group_check=True)
            else:
                lg = ps_lg_pool.tile([128, NEXP], F32)
                for k in range(KC):
                    last = k == KC - 1
                    nc.tensor.matmul(lg[:], xt_hi[:, k, :], w_sb[:, k, 0, :],
                                     start=(k == 0), stop=False)
                    nc.tensor.matmul(lg[:], xt_hi[:, k, :], w_sb[:, k, 1, :],
                                     start=False, stop=False)
                    nc.tensor.matmul(lg[:], xt_lo[:, k, :], w_sb[:, k, 0, :],
                                     start=False, stop=last)

            # combine partial logits into SBUF (only one PSUM input per op)
            logits = st_pool.tile([128, NEXP], F32, tag="logits")
            if mm_mode == "bf16x2p":
                t1 = st_pool.tile([128, NEXP], F32, tag="t1")
                nc.scalar.copy(t1[:], lg[:, NEXP : 2 * NEXP])
                if psum_disjoint:
                    t2 = st_pool.tile([128, NEXP], F32, tag="t2")
                    nc.vector.tensor_tensor(t2[:], lg[:, 0:NEXP], t1[:],
                                            op=mybir.AluOpType.add)
                    nc.vector.tensor_tensor(logits[:], lg[:, 2 * NEXP : 3 * NEXP], t2[:],
                                            op=mybir.AluOpType.add)
                else:
                    nc.vector.tensor_tensor(logits[:], lg[:, 0:NEXP], t1[:],
                                            op=mybir.AluOpType.add)
            else:
                nc.vector.tensor_copy(logits[:], lg[:, 0:NEXP])

            # routing phase (selection on unnormalized exp scores; monotone)
            escore = st_pool.tile([128, NEXP], F32, tag="escore")
            ssum = rt_pool.tile([128, 1], F32, tag="ssum")
            nc.scalar.activation(
                escore[:], logits[:], mybir.ActivationFunctionType.Exp,
                bias=0.0, scale=1.0, accum_out=ssum[:],
            )
            rec = rt_pool.tile([128, 1], F32, tag="rec")
            nc.vector.reciprocal(rec[:], ssum[:])
            gs = rt_pool.tile([128, NGROUP], F32, tag="gs")
            nc.vector.tensor_reduce(
                gs[:], escore[:].rearrange("p (g e) -> p g e", e=EPG),
                axis=mybir.AxisListType.X, op=mybir.AluOpType.max,
            )
            g8 = rt_pool.tile([128, 8], F32, tag="g8")
            nc.vector.max(out=g8[:], in_=gs[:])
            gmask = rt_pool.tile([128, NGROUP], F32, tag="gmask")
            nc.vector.tensor_scalar(
                gmask[:], gs[:], g8[:, TOPK_GROUP - 1 : TOPK_GROUP], None,
                op0=mybir.AluOpType.is_ge,
            )
            masked = st_pool.tile([128, NEXP], F32, tag="masked")
            nc.vector.tensor_tensor(
                masked[:].rearrange("p (g e) -> p g e", e=EPG),
                escore[:].rearrange("p (g e) -> p g e", e=EPG),
                gmask[:].to_broadcast([128, NGROUP, EPG]),
                op=mybir.AluOpType.mult,
            )
            top8 = rt_pool.tile([128, 8], F32, tag="top8")
            nc.vector.max(out=top8[:], in_=masked[:])
            o6 = rt_pool.tile([128, TOPK], F32, tag="o6")
            nc.vector.tensor_scalar(
                o6[:], top8[:, 0:TOPK], rec[:], SCALE,
                op0=mybir.AluOpType.mult, op1=mybir.AluOpType.mult,
            )
            # out-DMA on the ACT HWDGE ring: keeps the SP ring a pure input
            # prefetch stream (its sequencer would otherwise head-of-line
            # block the next x tile behind this DMA's wait on o6)
            nc.scalar.dma_start(out_dram[t * PT : (t + 1) * PT, :], o6[:])

    nc.compile()
    return nc


def prep_w(kernel_w: np.ndarray, mm_mode: str = MM_MODE) -> np.ndarray:
    w = np.asarray(kernel_w, dtype=np.float32)
    whi = w.astype(ml_dtypes.bfloat16)
    wlo = (w - whi.astype(np.float32)).astype(ml_dtypes.bfloat16)
    wb = np.stack([whi, wlo])  # [2, NEXP, HIDDEN]
    # [2, NEXP, HIDDEN] -> [HIDDEN, 2, NEXP] -> [KC, 128, 2, NEXP] -> [128, KC, 2, NEXP]
    arr = np.ascontiguousarray(
        wb.transpose(2, 0, 1).reshape(KC, 128, 2, NEXP).transpose(1, 0, 2, 3)
    )
    if mm_mode == "bf16x2p":
        return arr.reshape(128, KC, 2 * NEXP)
    return arr


def prep_x(x: np.ndarray, mm_mode: str = MM_MODE) -> np.ndarray:
    if mm_mode == "bf16x3":
        return np.ascontiguousarray(x, dtype=np.float32)
    x = np.asarray(x, dtype=np.float32)
    T = x.shape[0]
    xhi = x.astype(ml_dtypes.bfloat16)
    xlo = (x - xhi.astype(np.float32)).astype(ml_dtypes.bfloat16)
    X = np.stack([xhi, xlo])  # [2, T, H]
    # [j, t, c, k, p] -> [t, p, j, k, c]: DRAM row t*128+p (partition = hidden-in-chunk),
    # free dims [j, k, c] with c = token-in-tile
    X = X.reshape(2, T // PT, PT, KC, 128).transpose(1, 4, 0, 3, 2)
    return np.ascontiguousarray(X.reshape(T, 2, KC, 128))


def run(hidden_states: np.ndarray, kernel_w: np.ndarray, mm_mode: str = MM_MODE, **spmd_kwargs):
    x = prep_x(hidden_states, mm_mode)
    w_arr = prep_w(kernel_w, mm_mode)
    nc = build_nc(TPC, mm_mode=mm_mode)
    in_maps = [
        {"x": x[i * TPC : (i + 1) * TPC], "w": w_arr} for i in range(NCORES)
    ]
    res = run_bass_kernel_spmd(nc, in_maps, list(range(NCORES)), **spmd_kwargs)
    out = np.concatenate([res.results[i]["out"] for i in range(NCORES)], axis=0)
    return out, res


def kernel(hidden_states: np.ndarray, kernel: np.ndarray) -> np.ndarray:
    return run(hidden_states, kernel)[0]


# revision 24
# speedup vs baseline: 1.0128x; 1.0128x over previous
"""MoE gate routing kernel (DeepSeek-V2-style group-limited top-k) for 8x TRN2 NeuronCores.

Problem: nn_MoEGate_13907104105110
  hidden_states [32768, 5120] fp32, gate weight [160, 5120] fp32
  logits = x @ W.T ; scores = softmax(logits)
  group-limited greedy top-k: 8 groups of 20 experts, keep top-3 groups by
  group max score, then top-6 scores of the kept groups, scaled by 16.0.
  Output: [32768, 6] fp32 (top-6 weights, descending).

Sharding: data-parallel over tokens; 4096 tokens per core, W replicated.

Modes:
  bf16x2p (default): host pre-transposes x and splits x,W into bf16 hi/lo.
    Per 128-token tile the device only does: one 2.62MB DMA, then per
    128-hidden chunk two matmuls with the x chunks stationary:
      MM1: xhi_k (stationary) x [Whi_k | Wlo_k]  -> PSUM[:, 0:320]   (N=320)
      MM2: xlo_k (stationary) x Whi_k            -> PSUM[:, 320:480] (N=160)
    logits = P[:,0:160] + P[:,160:320] + P[:,320:480]  (error ~2^-18, needed:
    pure-bf16 / fp32r logits flip near-ties in the top-k -> rel err ~1).
  bf16x3p: host pre-transposed, classic 3-MM hi/lo into one [160] region.
  bf16x3: on-device PE transpose variant (original baseline path).

Routing per tile (no max-subtraction: |logits| <~ 8 for this distribution, and
softmax selection is monotone so top-k is done on unnormalized exp scores):
  escore = Exp(logits) with accum ssum (ACT) -> rec = 1/ssum (DVE)
  group max over [128, 8, 20] -> top-8 -> 3rd value as group threshold
  -> mask groups (mult; escore > 0 so zeroing is safe) -> top-8 of masked
  -> out = top6 * rec * 16.0.
"""

import sys

if "/opt/trn_rl_repo" not in sys.path:
    sys.path.insert(0, "/opt/trn_rl_repo")

from contextlib import ExitStack

import ml_dtypes
import numpy as np

import concourse.bass as bass
import concourse.mybir as mybir
from concourse import bacc
from concourse import tile
from concourse.bass_utils import run_bass_kernel_spmd
from concourse.masks import make_identity

TOKENS = 32768
HIDDEN = 5120
NEXP = 160
TOPK = 6
NGROUP = 8
EPG = NEXP // NGROUP  # 20 experts per group
TOPK_GROUP = 3
SCALE = 16.0
NCORES = 8
TPC = TOKENS // NCORES  # 4096 tokens per core
PT = 128  # tokens per tile
KC = HIDDEN // 128  # 40 contraction chunks

F32 = mybir.dt.float32
BF16 = mybir.dt.bfloat16

MM_MODE = "bf16x2p"


def build_nc(tokens_per_core: int = TPC, mm_mode: str = MM_MODE, repeat: int = 1,
             hw_repeat: int = 1, bufs: dict | None = None,
             probe: str | None = None, psum_disjoint: bool = False,
             dma_split: bool = False, lo_n: int = NEXP) -> bass.Bass:
    # probe="dma": k-loop only does chunk 0 (measures DMA/routing side)
    # probe="pe":  every tile reuses tile 0's data (measures PE side)
    # psum_disjoint: lo@whi accumulates into its own PSUM region [320:480]
    # dma_split: issue hi/lo halves of the x tile on separate HWDGE rings
    B = {"x": 4, "xt": 2, "ps_tr": 4, "ps_lg": 4, "rt": 3, "st": 3}
    B.update(bufs or {})
    nt = tokens_per_core // PT
    nc = bacc.Bacc("TRN2", target_bir_lowering=False, debug=False)

    pretr = mm_mode in ("bf16x2p", "bf16x3p")
    if pretr:
        # host-pretransposed hi/lo: row t*128+p (p = hidden-in-chunk),
        # free dims [j(hi/lo), kchunk, token-in-tile]
        x_dram = nc.dram_tensor("x", [tokens_per_core, 2, KC, 128], BF16, kind="ExternalInput")
    else:
        x_dram = nc.dram_tensor("x", [tokens_per_core, HIDDEN], F32, kind="ExternalInput")
    if mm_mode == "bf16x2p":
        w_shape = [128, KC, 2 * NEXP]  # [Whi_k | Wlo_k] along free
    else:
        w_shape = [128, KC, 2, NEXP]
    w_dram = nc.dram_tensor("w", w_shape, BF16, kind="ExternalInput")
    out_dram = nc.dram_tensor("out", [tokens_per_core, TOPK], F32, kind="ExternalOutput")

    with tile.TileContext(nc) as tc, ExitStack() as ctx:
        const_pool = ctx.enter_context(tc.tile_pool(name="const", bufs=1))
        x_pool = ctx.enter_context(tc.tile_pool(name="x", bufs=B["x"]))
        ps_lg_pool = ctx.enter_context(tc.tile_pool(name="ps_lg", bufs=B["ps_lg"], space="PSUM"))
        rt_pool = ctx.enter_context(tc.tile_pool(name="rt", bufs=B["rt"]))
        st_pool = ctx.enter_context(tc.tile_pool(name="st", bufs=B["st"]))
        if not pretr:
            xt_pool = ctx.enter_context(tc.tile_pool(name="xt", bufs=B["xt"]))
            ps_tr_pool = ctx.enter_context(tc.tile_pool(name="ps_tr", bufs=B["ps_tr"], space="PSUM"))

        w_sb = const_pool.tile(w_shape, BF16)
        nc.sync.dma_start(w_sb[:], w_dram[:])
        if not pretr:
            ident = const_pool.tile([128, 128], F32)
            make_identity(nc, ident[:])

        xts0 = None
        if pretr and probe == "pe":
            xts0 = const_pool.tile([128, 2, KC, 128], BF16)
            nc.sync.dma_start(xts0[:], x_dram[0:PT, :, :, :])

        loop_ctx = ExitStack()
        if hw_repeat > 1:
            # hardware loop re-running the whole per-core pipeline; used by
            # test.py to amortize per-dispatch tunnel latency out of the
            # timing (each iteration is a complete kernel execution).
            loop_ctx.enter_context(
                tc.For_i(0, hw_repeat, 1,
                         hint_engines=(mybir.EngineType.PE, mybir.EngineType.DVE,
                                       mybir.EngineType.Activation, mybir.EngineType.SP))
            )
        ctx.enter_context(loop_ctx)

        for t in [i for _ in range(repeat) for i in range(nt)]:
            if pretr:
                if probe == "pe":
                    xts = xts0
                else:
                    xts = x_pool.tile([128, 2, KC, 128], BF16)
                    if dma_split:
                        nc.sync.dma_start(xts[:, 0], x_dram[t * PT : (t + 1) * PT, 0])
                        nc.scalar.dma_start(xts[:, 1], x_dram[t * PT : (t + 1) * PT, 1])
                    else:
                        nc.sync.dma_start(xts[:], x_dram[t * PT : (t + 1) * PT, :, :, :])
                xt_hi = xts[:, 0]
                xt_lo = xts[:, 1]
            else:
                x_sb = x_pool.tile([128, HIDDEN], F32)
                nc.sync.dma_start(x_sb[:], x_dram[t * PT : (t + 1) * PT, :])
                # transpose phase: x tile -> xT [128 hidden, KC, 128 tokens]
                xt_hi_t = xt_pool.tile([128, KC, 128], BF16, tag="xt_hi")
                xt_lo_t = xt_pool.tile([128, KC, 128], BF16, tag="xt_lo")
                GK = 4  # transposed chunks per PSUM bank; one batched copy per group
                for g in range(KC // GK):
                    xt_ps = ps_tr_pool.tile([128, GK, 128], F32)
                    for j in range(GK):
                        k = g * GK + j
                        nc.tensor.transpose(
                            xt_ps[:, j, :], x_sb[:, k * 128 : (k + 1) * 128], ident[:]
                        )
                    ks = slice(g * GK, (g + 1) * GK)
                    # hi = bf16(xT); lo = bf16(xT - hi)
                    nc.scalar.copy(xt_hi_t[:, ks, :], xt_ps[:])
                    nc.vector.tensor_sub(xt_lo_t[:, ks, :], xt_ps[:], xt_hi_t[:, ks, :])
                xt_hi = xt_hi_t[:]
                xt_lo = xt_lo_t[:]

            # matmul phase
            krange = [0] if probe == "dma" else list(range(KC))
            if mm_mode == "bf16x2p":
                # lg[:, 0:160] accumulates hi@whi; lg[:, 160:320] accumulates
                # hi@wlo (from the N=320 stream) PLUS lo@whi (second matmul
                # accumulating into the same region). PSUM accumulation is
                # per-element has_written on HW, so interleaved groups are
                # fine; skip_group_check silences the interp-only zero-region
                # bookkeeping which can't express overlapping groups.
                nl = 3 * NEXP if psum_disjoint else 2 * NEXP
                lg = ps_lg_pool.tile([128, nl], F32)
                for k in krange:
                    nc.tensor.matmul(lg[:, 0 : 2 * NEXP], xt_hi[:, k, :], w_sb[:, k, :],
                                     start=(k == 0), stop=(k == krange[-1]),
                                     skip_group_check=True)
                    if psum_disjoint:
                        nc.tensor.matmul(lg[:, 2 * NEXP : 3 * NEXP], xt_lo[:, k, :],
                                         w_sb[:, k, 0:NEXP],
                                         start=(k == 0), stop=(k == krange[-1]),
                                         skip_group_check=True)
                    else:
                        # lo stream width lo_n >= NEXP: columns beyond NEXP
                        # stream wlo and add genuine xlo@wlo terms into
                        # [NEXP:lo_n]; widths >=256 keep the stream long
                        # enough to hide the next chunk's LDWEIGHTS.
                        nc.tensor.matmul(lg[:, 0:lo_n], xt_lo[:, k, :],
                                         w_sb[:, k, 0:lo_n],
                                         start=False, stop=(k == krange[-1]),
                                         skip_group_check=True)
            else:
                lg = ps_lg_pool.tile([128, NEXP], F32)
                for k in range(KC):
                    last = k == KC - 1
                    nc.tensor.matmul(lg[:], xt_hi[:, k, :], w_sb[:, k, 0, :],
                                     start=(k == 0), stop=False)
                    nc.tensor.matmul(lg[:], xt_hi[:, k, :], w_sb[:, k, 1, :],
                                     start=False, stop=False)
                    nc.tensor.matmul(lg[:], xt_lo[:, k, :], w_sb[:, k, 0, :],
                                     start=False, stop=last)

            # combine partial logits into SBUF (only one PSUM input per op)
            logits = st_pool.tile([128, NEXP], F32, tag="logits")
            if mm_mode == "bf16x2p":
                t1 = st_pool.tile([128, NEXP], F32, tag="t1")
                nc.scalar.copy(t1[:], lg[:, NEXP : 2 * NEXP])
                if psum_disjoint:
                    t2 = st_pool.tile([128, NEXP], F32, tag="t2")
                    nc.vector.tensor_tensor(t2[:], lg[:, 0:NEXP], t1[:],
                                            op=mybir.AluOpType.add)
                    nc.vector.tensor_tensor(logits[:], lg[:, 2 * NEXP : 3 * NEXP], t2[:],
                                            op=mybir.AluOpType.add)
                else:
                    nc.vector.tensor_tensor(logits[:], lg[:, 0:NEXP], t1[:],
                                            op=mybir.AluOpType.add)
            else:
                nc.vector.tensor_copy(logits[:], lg[:, 0:NEXP])

            # routing phase (selection on unnormalized exp scores; monotone)
            escore = st_pool.tile([128, NEXP], F32, tag="escore")
            ssum = rt_pool.tile([128, 1], F32, tag="ssum")
            nc.scalar.activation(
                escore[:], logits[:], mybir.ActivationFunctionType.Exp,
                bias=0.0, scale=1.0, accum_out=ssum[:],
            )
            rec = rt_pool.tile([128, 1], F32, tag="rec")
            nc.vector.reciprocal(rec[:], ssum[:])
            gs = rt_pool.tile([128, NGROUP], F32, tag="gs")
            nc.vector.tensor_reduce(
                gs[:], escore[:].rearrange("p (g e) -> p g e", e=EPG),
                axis=mybir.AxisListType.X, op=mybir.AluOpType.max,
            )
            g8 = rt_pool.tile([128, 8], F32, tag="g8")
            nc.vector.max(out=g8[:], in_=gs[:])
            gmask = rt_pool.tile([128, NGROUP], F32, tag="gmask")
            nc.vector.tensor_scalar(
                gmask[:], gs[:], g8[:, TOPK_GROUP - 1 : TOPK_GROUP], None,
                op0=mybir.AluOpType.is_ge,
            )
            masked = st_pool.tile([128, NEXP], F32, tag="masked")
            nc.vector.tensor_tensor(
                masked[:].rearrange("p (g e) -> p g e", e=EPG),
                escore[:].rearrange("p (g e) -> p g e", e=EPG),
                gmask[:].to_broadcast([128, NGROUP, EPG]),
                op=mybir.AluOpType.mult,
            )
            top8 = rt_pool.tile([128, 8], F32, tag="top8")
            nc.vector.max(out=top8[:], in_=masked[:])
            o6 = rt_pool.tile([128, TOPK], F32, tag="o6")
            nc.vector.tensor_scalar(
                o6[:], top8[:, 0:TOPK], rec[:], SCALE,
                op0=mybir.AluOpType.mult, op1=mybir.AluOpType.mult,
            )
            # out-DMA on the ACT HWDGE ring: keeps the SP ring a pure input
            # prefetch stream (its sequencer would otherwise head-of-line
            # block the next x tile behind this DMA's wait on o6)
            nc.scalar.dma_start(out_dram[t * PT : (t + 1) * PT, :], o6[:])

    nc.compile()
    return nc


def prep_w(kernel_w: np.ndarray, mm_mode: str = MM_MODE) -> np.ndarray:
    w = np.asarray(kernel_w, dtype=np.float32)
    whi = w.astype(ml_dtypes.bfloat16)
    wlo = (w - whi.astype(np.float32)).astype(ml_dtypes.bfloat16)
    wb = np.stack([whi, wlo])  # [2, NEXP, HIDDEN]
    # [2, NEXP, HIDDEN] -> [HIDDEN, 2, NEXP] -> [KC, 128, 2, NEXP] -> [128, KC, 2, NEXP]
    arr = np.ascontiguousarray(
        wb.transpose(2, 0, 1).reshape(KC, 128, 2, NEXP).transpose(1, 0, 2, 3)
    )
    if mm_mode == "bf16x2p":
        return arr.reshape(128, KC, 2 * NEXP)
    return arr


def prep_x(x: np.ndarray, mm_mode: str = MM_MODE) -> np.ndarray:
    if mm_mode == "bf16x3":
        return np.ascontiguousarray(x, dtype=np.float32)
    x = np.asarray(x, dtype=np.float32)
    T = x.shape[0]
    xhi = x.astype(ml_dtypes.bfloat16)
    xlo = (x - xhi.astype(np.float32)).astype(ml_dtypes.bfloat16)
    X = np.stack([xhi, xlo])  # [2, T, H]
    # [j, t, c, k, p] -> [t, p, j, k, c]: DRAM row t*128+p (partition = hidden-in-chunk),
    # free dims [j, k, c] with c = token-in-tile
    X = X.reshape(2, T // PT, PT, KC, 128).transpose(1, 4, 0, 3, 2)
    return np.ascontiguousarray(X.reshape(T, 2, KC, 128))


def run(hidden_states: np.ndarray, kernel_w: np.ndarray, mm_mode: str = MM_MODE, **spmd_kwargs):
    x = prep_x(hidden_states, mm_mode)
    w_arr = prep_w(kernel_w, mm_mode)
    nc = build_nc(TPC, mm_mode=mm_mode)
    in_maps = [
        {"x": x[i * TPC : (i + 1) * TPC], "w": w_arr} for i in range(NCORES)
    ]
    res = run_bass_kernel_spmd(nc, in_maps, list(range(NCORES)), **spmd_kwargs)
    out = np.concatenate([res.results[i]["out"] for i in range(NCORES)], axis=0)
    return out, res


def kernel(hidden_states: np.ndarray, kernel: np.ndarray) -> np.ndarray:
    return run(hidden_states, kernel)[0]
